# revision 11
# baseline (speedup 1.0000x reference)
"""Trainium2 Bass kernel for nn_CrossAttention (B=4, C=256, H=W=64, RC=32).

Sharding: 8 cores = (batch b in 0..3) x (query-pixel half in 0..1).
Each core gets x[b][:, nloc] (256 x 2048), the full context[b] (256 x 4096),
and replicated weights; it computes out[b][:, nloc].T (2048 x 256).
No collectives: k/v are recomputed on both cores sharing a batch item.

v4 design (vs v3 at 128.3us HW, v2 at 128.7us):
  The kernel is ScalarE-bound: exp over NLOC*M = 8.4M logits is ~65us of
  Activation-engine time (measured logits+exp alone = 65.1us). The v2/v3
  structures lost ~60us because (a) with only 2 logits half-tiles (4 PSUM
  banks) the exp->logits PSUM-WAR ping-pong has ~0.2us/half-step of slack,
  so every att@v matmul placed in the in-order PE queue added straight to
  the period, and (b) projections/epilogue serialization.
  v4 restructure:
  - logits half-tiles rotate over THREE tags (6 banks): tag reuse distance
    1.5 steps, so the next logits never wait on the previous exp's read.
  - att@v runs ONE FULL STRIP behind in a single 2-bank accumulator pair:
    strip s's n-tiles j0,j1 accumulate during the first 4 steps of strip
    s+1 (2 groups/step), then a half-epilogue frees the banks, then j2,j3
    in the next 4 steps. At-tiles buffer ~12 steps (attn bufs=14).
  - strip 0 interleaves q/k/v projections (pq+pk share 1 bank; pv double-
    buffers in the 2 av banks, 2 m-tiles per pv tile); per-group k/v
    emitted one group ahead of their logits.
  - epilogue fused to one scalar_tensor_tensor per n-tile:
    res = (av * gamma/(16*rowsum)) + (xT + gamma*bv), xgv precomputed on
    GPSIMD. x residual input is bf16; output is bf16.
  - input DMAs spread over SP/DVE/Pool queues (none on ACT), 512-col
    first pieces so group-0 projections start ~2us in.
"""

import math
import os
from contextlib import ExitStack

import numpy as np

import concourse.bass as bass
import concourse.mybir as mybir
import concourse.tile as tile
from concourse import bacc
from concourse.bass_utils import run_bass_kernel_spmd

B, C, RC = 4, 256, 32
H = W = 64
NPIX = H * W          # 4096 query pixels per batch item
M = NPIX              # context pixels
NLOC = NPIX // 2      # query pixels per core
P = 128
N_CORES = 8
SW = 512              # attention strip width (n cols per strip)
NSTRIPS = NLOC // SW  # 4
MT = M // P           # 32 m-tiles
NGRP = MT // 4        # 8 groups of 4 m-tiles
NSTEP = NSTRIPS * NGRP  # 32 (strip, group) steps
WS = 16.0             # host-side weight scale for fp8
F32 = mybir.dt.float32
BF = mybir.dt.bfloat16
F8 = mybir.dt.float8e4
SCALE = 1.0 / math.sqrt(RC) / (WS * WS)
MULT = mybir.AluOpType.mult
ADD = mybir.AluOpType.add

_CACHE = {}
_ABL = os.environ.get("ABL", "")


def build_nc(reps=1, loop_n=0):
    nc = bacc.Bacc("TRN2", target_bir_lowering=False, debug=False)
    x8d = nc.dram_tensor("x8", [C, NLOC], F8, kind="ExternalInput").ap()
    xTd = nc.dram_tensor("xT", [NLOC, C], BF, kind="ExternalInput").ap()
    cfd = nc.dram_tensor("cf8", [C, M], F8, kind="ExternalInput").ap()
    # packed weights (C, 416): [Wq4T | WkT | WvT] fp8, all pre-scaled x16
    wd = nc.dram_tensor("Wpack", [C, 416], F8, kind="ExternalInput").ap()
    # packed per-partition constants (128, 259): [bq4 | bk4 | gamma | bv_bc]
    bd = nc.dram_tensor("Bpack", [P, 259], F32, kind="ExternalInput").ap()
    o_dram = nc.dram_tensor("out", [NLOC, C], BF, kind="ExternalOutput").ap()

    with tile.TileContext(nc) as tc:
        if loop_n:
            # hardware loop: same body executed loop_n times (for timing)
            with tc.For_i(0, loop_n, 1):
                _emit(tc, x8d, xTd, cfd, wd, bd, o_dram)
        else:
            for _ in range(reps):
                _emit(tc, x8d, xTd, cfd, wd, bd, o_dram)
    nc.compile()
    return nc


def _emit(tc, x8d, xTd, cfd, wd, bd, o_dram):
    nc = tc.nc

    with ExitStack() as ctx:
        const = ctx.enter_context(tc.tile_pool(name="const", bufs=1))

        # ---- input DMAs: SP/DVE/Pool queues only (ACT stays exp-only) ----
        wall = const.tile([P, 2, 416], F8)       # [c_in_chunk, ci, col]
        nc.sync.dma_start(
            out=wall,
            in_=bass.AP(tensor=wd.tensor, offset=wd.offset,
                        ap=[[416, P], [416 * P, 2], [1, 416]]),
        )
        wq4 = wall[:, :, 0:P]
        wk = wall[:, :, P:P + RC]
        wv = wall[:, :, P + RC:416]
        cf8 = const.tile([P, 2, M], F8)

        def cf_dma(eng, c0, c1):
            eng.dma_start(
                out=cf8[:, :, c0:c1],
                in_=bass.AP(tensor=cfd.tensor, offset=cfd.offset + c0,
                            ap=[[M, P], [M * P, 2], [1, c1 - c0]]),
            )

        xf8 = const.tile([P, 2, NLOC], F8)       # [c_in_chunk, ci, n]

        def xf_dma(eng, a, b):
            eng.dma_start(
                out=xf8[:, :, a:b],
                in_=bass.AP(tensor=x8d.tensor, offset=x8d.offset + a,
                            ap=[[NLOC, P], [NLOC * P, 2], [1, b - a]]),
            )

        cf_dma(nc.sync, 0, 512)
        xf_dma(nc.sync, 0, 512)
        xf_dma(nc.sync, 512, NLOC)
        cf_dma(nc.sync, 1536, 2048)
        cf_dma(nc.sync, 3584, 4096)
        cf_dma(nc.scalar, 512, 1024)
        cf_dma(nc.scalar, 1024, 1536)
        bpack = const.tile([P, 259], F32)
        nc.gpsimd.dma_start(out=bpack, in_=bd)
        cf_dma(nc.gpsimd, 2048, 2560)
        cf_dma(nc.gpsimd, 2560, 3072)
        cf_dma(nc.gpsimd, 3072, 3584)
        bq4_sb = bpack[:, 0:1]
        bk4_sb = bpack[:, 1:2]
        gamma_bc = bpack[:, 2:3]
        bv_bc = bpack[:, 3:259]
        xbT = const.tile([P, NLOC // P, C], BF)  # [n_in_tile, nt, d]
        for tb in range(2):
            t0 = tb * 8
            nc.gpsimd.dma_start(
                out=xbT[:, t0:t0 + 8, :],
                in_=bass.AP(tensor=xTd.tensor, offset=xTd.offset + t0 * P * C,
                            ap=[[C, P], [P * C, 8], [1, C]]),
            )

        # gamma/16 (fp8 v-scale fold); xgv = xT + gamma*bv (epilogue adduct)
        gamma16 = const.tile([P, 1], F32)
        nc.vector.tensor_scalar_mul(gamma16, gamma_bc, 1.0 / WS)
        gvb = const.tile([P, C], F32)
        nc.gpsimd.tensor_scalar_mul(gvb, bv_bc, gamma_bc)
        xgv = const.tile([P, NLOC // P, C], F32)

        # ---- persistent SBUF operands -----------------------------------
        q4 = const.tile([P, NLOC], BF)           # q replicated on 4 part groups
        k4 = const.tile([P, NGRP, P], BF)        # m-tile 4g+i at partitions 32i
        vT = const.tile([P, MT, 272], F8)        # [m_in_tile, mt, d]; col 256=1
        nc.gpsimd.memset(vT[:, :, 257:258], 0.0)
        nc.gpsimd.memset(vT[:, :, 256:257], 1.0)

        ATTN_BUFS = int(os.environ.get("ATTN_BUFS", "14"))
        ATW = int(os.environ.get("ATW", "512"))
        attn = ctx.enter_context(tc.tile_pool(name="attn", bufs=ATTN_BUFS))
        eps = ctx.enter_context(tc.tile_pool(name="eps", bufs=4))
        outp = ctx.enter_context(tc.tile_pool(name="outp", bufs=2))

        at_tiles = {}
        res2_by_strip = {}
        at_static = []
        if "avstatic" in _ABL:
            ats0 = const.tile([P, 4, SW], F8, name="ats0")
            nc.gpsimd.memset(ats0, 0.002)
            at_static.append(ats0)

        with tc.tile_pool(name="psL", bufs=1, space="PSUM") as psL:

            def logits_exp(T):
                s, g = divmod(T, NGRP)
                nsl = slice(s * SW, (s + 1) * SW)
                at = attn.tile([P, 4, ATW], F8, tag="at", name="at")
                for h in range(2):
                    # three rotating half-tiles: reuse distance 1.5 steps,
                    # so these never wait on the previous step's exp read
                    tg = (2 * T + h) % 3
                    pl = psL.tile([P, 2, SW], F32, tag=f"pl{tg}", name=f"pl{tg}")
                    for i in (2 * h, 2 * h + 1):
                        nc.tensor.matmul(
                            pl[:, i % 2, :],
                            k4[32 * i:32 * (i + 1), g, :],
                            q4[32 * i:32 * (i + 1), nsl],
                            start=True, stop=True,
                            tile_position=(32 * i, 0),
                        )
                    nc.scalar.activation(
                        out=at[:, 2 * h:2 * h + 2, 0:SW], in_=pl[:, :, :],
                        func=mybir.ActivationFunctionType.Exp, scale=SCALE,
                    )
                at_tiles[T] = at

            def xgv_prep(T):
                t = T - 4
                if 0 <= t < NLOC // P:
                    nc.gpsimd.tensor_add(xgv[:, t, :], xbT[:, t, :], gvb)

            # ---- strip 0: projections interleaved with logits/exp -------
            with tc.tile_pool(name="psP", bufs=1, space="PSUM") as psP, \
                 tc.tile_pool(name="psV", bufs=1, space="PSUM") as psV:

                def qchunk(sq):
                    sl = slice(sq * 512, (sq + 1) * 512)
                    pq = psP.tile([P, 512], F32, tag="pp")
                    nc.tensor.matmul(
                        pq, wq4[:, :, :], xf8[:, :, sl],
                        start=True, stop=True,
                        perf_mode=mybir.MatmulPerfMode.DoubleRow,
                    )
                    nc.vector.tensor_scalar_add(q4[:, sl], pq, bq4_sb)

                def kproj(g):
                    # 4-way column-tiled: m-tile 4g+i -> partitions 32i
                    pk = psP.tile([P, 512], F32, tag="pp")
                    for i in range(4):
                        for ci in range(2):
                            msl = slice((4 * g + i) * P, (4 * g + i + 1) * P)
                            nc.tensor.matmul(
                                pk[32 * i:32 * (i + 1), 0:P],
                                wk[:, ci, :], cf8[:, ci, msl],
                                start=(ci == 0),
                                stop=(i == 3 and ci == 1),
                                skip_group_check=True,
                                tile_position=(0, 32 * i),
                            )
                    nc.vector.tensor_scalar_add(k4[:, g, :], pk[:, 0:P], bk4_sb)

                def vproj_half(g, half):
                    # two m-tiles per pv tile (2x256 cols in one bank);
                    # the two halves are emitted at opposite ends of a step
                    # so the PSUM-WAR on the single bank never stalls PE
                    pv = psV.tile([P, 512], F32, tag="pv")
                    mt0 = 4 * g + 2 * half
                    for q in range(2):
                        msl = slice((mt0 + q) * P, (mt0 + q + 1) * P)
                        nc.tensor.matmul(
                            pv[:, q * C:(q + 1) * C],
                            cf8[:, :, msl], wv[:, :, :],
                            start=True, stop=True,
                            perf_mode=mybir.MatmulPerfMode.DoubleRow,
                        )
                    # copies after both matmuls: DVE-read + PE-write of the
                    # same PSUM bank would otherwise be serialized (fatal on
                    # HW if concurrent)
                    for q in range(2):
                        nc.vector.tensor_copy(
                            vT[:, mt0 + q, 0:256], pv[:, q * C:(q + 1) * C]
                        )

                if "noproj" in _ABL:
                    nc.gpsimd.memset(q4, 0.001)
                    nc.gpsimd.memset(k4, 0.001)
                    nc.gpsimd.memset(vT[:, :, 0:256], 0.001)
                    for g in range(NGRP):
                        logits_exp(g)
                        xgv_prep(g)
                else:
                    qchunk(0)
                    kproj(0)
                    vproj_half(0, 0)
                    for g in range(NGRP):
                        vproj_half(g, 1)
                        logits_exp(g)
                        if g < 3:
                            qchunk(g + 1)
                        if g < NGRP - 1:
                            kproj(g + 1)
                            vproj_half(g + 1, 0)
                        xgv_prep(g)

            # ---- strips 1-3 + one-strip-behind att@v in a 2-bank pair ---
            with tc.tile_pool(name="psAV", bufs=1, space="PSUM") as psAV:
                av_cur = {}

                def av_chunk(s2, pair, g2):
                    # pair 0 -> n-tiles (0,1) of strip s2; pair 1 -> (2,3)
                    if g2 == 0:
                        av_cur["t"] = [
                            psAV.tile([P, 512], F32, tag=f"av{u}", name=f"av{u}")
                            for u in range(2)
                        ]
                    ats = at_tiles[s2 * NGRP + g2]
                    if pair == 1 and g2 == NGRP - 1:
                        del at_tiles[s2 * NGRP + g2]
                    if "avstatic" in _ABL:
                        ats = at_static[0]
                    for tl in range(2):
                        mtt = 4 * g2 + 2 * tl
                        for u in range(2):
                            j = 2 * pair + u
                            nc.tensor.matmul(
                                av_cur["t"][u][:, 0:258],
                                ats[:, 2 * tl:2 * tl + 2, j * P:(j + 1) * P],
                                vT[:, mtt:mtt + 2, 0:258],
                                start=(g2 == 0 and tl == 0),
                                stop=(g2 == NGRP - 1 and tl == 1),
                                perf_mode=mybir.MatmulPerfMode.DoubleRow,
                            )

                def epi_pair(s2, pair):
                    if "noepi" in _ABL:
                        sink = eps.tile([P, 2], F32, tag="sink", name="sink")
                        for u in range(2):
                            nc.vector.tensor_copy(
                                sink[:, u:u + 1], av_cur["t"][u][:, 256:257])
                        return
                    if pair == 0:
                        res2_by_strip[s2] = outp.tile([P, 4, C], BF, tag="res2", name="res2")
                    res2 = res2_by_strip[s2]
                    for u in range(2):
                        j = 2 * pair + u
                        t = 4 * s2 + j
                        rec = eps.tile([P, 1], F32, tag="rec", name="rec")
                        nc.vector.reciprocal(rec, av_cur["t"][u][:, 256:257])
                        recg = eps.tile([P, 1], F32, tag="recg", name="recg")
                        nc.vector.tensor_scalar_mul(recg, rec, gamma16)
                        nc.vector.scalar_tensor_tensor(
                            res2[:, j, :], av_cur["t"][u][:, 0:256], recg,
                            xgv[:, t, :], MULT, ADD,
                        )
                    if pair == 1:
                        nc.sync.dma_start(
                            out=bass.AP(
                                tensor=o_dram.tensor,
                                offset=o_dram.offset + 4 * s2 * P * C,
                                ap=[[C, P], [P * C, 4], [1, C]],
                            ),
                            in_=res2_by_strip.pop(s2),
                        )

                def av_slot(sA, w):
                    pair, ww = divmod(w, 4)
                    av_chunk(sA, pair, 2 * ww)
                    av_chunk(sA, pair, 2 * ww + 1)
                    if ww == 3:
                        epi_pair(sA, pair)

                for T in range(NGRP, NSTEP):
                    logits_exp(T)
                    if "noav" not in _ABL:
                        av_slot(T // NGRP - 1, T % NGRP)
                    xgv_prep(T)
                if "noav" not in _ABL:
                    for w in range(NGRP):      # drain strip 3
                        av_slot(NSTRIPS - 1, w)


def _shard_inputs(x, context, Wq, bq, Wk, bk, Wv, bv, gamma):
    import ml_dtypes

    f8 = ml_dtypes.float8_e4m3
    bf16 = ml_dtypes.bfloat16
    xb = np.ascontiguousarray(np.asarray(x, dtype=np.float32)).reshape(B, C, NPIX)
    cb = np.ascontiguousarray(np.asarray(context, dtype=np.float32)).reshape(B, C, NPIX)
    WqT = np.asarray(Wq, dtype=np.float32).T * WS      # (C, RC)
    wpack = np.concatenate(
        [np.tile(WqT, (1, 4)),
         np.asarray(Wk, np.float32).T * WS,
         np.asarray(Wv, np.float32).T * WS], axis=1)
    bpack = np.concatenate(
        [np.tile(np.asarray(bq, np.float32) * WS, 4)[:, None],
         np.tile(np.asarray(bk, np.float32) * WS, 4)[:, None],
         np.full((P, 1), np.float32(np.asarray(gamma).reshape(-1)[0]), np.float32),
         np.tile(np.asarray(bv, np.float32)[None, :], (P, 1))], axis=1)
    shared = {
        "Wpack": np.ascontiguousarray(wpack.astype(f8)),
        "Bpack": np.ascontiguousarray(bpack.astype(np.float32)),
    }
    in_maps = []
    for core in range(N_CORES):
        b, half = core // 2, core % 2
        xs = xb[b][:, half * NLOC:(half + 1) * NLOC]
        m = dict(shared)
        m["x8"] = np.ascontiguousarray(xs.astype(f8))
        m["xT"] = np.ascontiguousarray(xs.T.astype(bf16))
        m["cf8"] = np.ascontiguousarray(cb[b].astype(f8))
        in_maps.append(m)
    return in_maps


def _gather(results):
    out = np.empty((B, C, NPIX), dtype=np.float32)
    for core in range(N_CORES):
        b, half = core // 2, core % 2
        out[b][:, half * NLOC:(half + 1) * NLOC] = (
            results[core]["out"].astype(np.float32).T
        )
    return out.reshape(B, C, H, W)


def run(inputs, trace=False, **kw):
    """Build (cached), run on the 8 NeuronCores, return (output, results)."""
    if "nc" not in _CACHE:
        _CACHE["nc"] = build_nc()
    nc = _CACHE["nc"]
    in_maps = _shard_inputs(**inputs)
    res = run_bass_kernel_spmd(
        nc, in_maps, core_ids=list(range(N_CORES)), trace=trace, **kw
    )
    return _gather(res.results), res


def kernel(**inputs) -> np.ndarray:
    out, _ = run(inputs, trace=False)
    return out


# revision 14
# speedup vs baseline: 1.0302x; 1.0302x over previous
"""Trainium2 Bass kernel for nn_CrossAttention (B=4, C=256, H=W=64, RC=32).

Sharding: 8 cores = (batch b in 0..3) x (query-pixel half in 0..1).
Each core gets x[b][:, nloc] (256 x 2048), the full context[b] (256 x 4096),
and replicated weights; it computes out[b][:, nloc].T (2048 x 256).
No collectives: k/v are recomputed on both cores sharing a batch item.

v4 design (vs v3 at 128.3us HW, v2 at 128.7us):
  The kernel is ScalarE-bound: exp over NLOC*M = 8.4M logits is ~65us of
  Activation-engine time (measured logits+exp alone = 65.1us). The v2/v3
  structures lost ~60us because (a) with only 2 logits half-tiles (4 PSUM
  banks) the exp->logits PSUM-WAR ping-pong has ~0.2us/half-step of slack,
  so every att@v matmul placed in the in-order PE queue added straight to
  the period, and (b) projections/epilogue serialization.
  v4 restructure:
  - logits half-tiles rotate over THREE tags (6 banks): tag reuse distance
    1.5 steps, so the next logits never wait on the previous exp's read.
  - att@v runs ONE FULL STRIP behind in a single 2-bank accumulator pair:
    strip s's n-tiles j0,j1 accumulate during the first 4 steps of strip
    s+1 (2 groups/step), then a half-epilogue frees the banks, then j2,j3
    in the next 4 steps. At-tiles buffer ~12 steps (attn bufs=14).
  - strip 0 interleaves q/k/v projections (pq+pk share 1 bank; pv double-
    buffers in the 2 av banks, 2 m-tiles per pv tile); per-group k/v
    emitted one group ahead of their logits.
  - epilogue fused to one scalar_tensor_tensor per n-tile:
    res = (av * gamma/(16*rowsum)) + (xT + gamma*bv), xgv precomputed on
    GPSIMD. x residual input is bf16; output is bf16.
  - input DMAs spread over SP/DVE/Pool queues (none on ACT), 512-col
    first pieces so group-0 projections start ~2us in.
"""

import math
import os
from contextlib import ExitStack

import numpy as np

import concourse.bass as bass
import concourse.mybir as mybir
import concourse.tile as tile
from concourse import bacc
from concourse.bass_utils import run_bass_kernel_spmd

B, C, RC = 4, 256, 32
H = W = 64
NPIX = H * W          # 4096 query pixels per batch item
M = NPIX              # context pixels
NLOC = NPIX // 2      # query pixels per core
P = 128
N_CORES = 8
SW = 512              # attention strip width (n cols per strip)
NSTRIPS = NLOC // SW  # 4
MT = M // P           # 32 m-tiles
NGRP = MT // 4        # 8 groups of 4 m-tiles
NSTEP = NSTRIPS * NGRP  # 32 (strip, group) steps
WS = 16.0             # host-side weight scale for fp8
F32 = mybir.dt.float32
BF = mybir.dt.bfloat16
F8 = mybir.dt.float8e4
SCALE = 1.0 / math.sqrt(RC) / (WS * WS)
MULT = mybir.AluOpType.mult
ADD = mybir.AluOpType.add

_CACHE = {}
_ABL = os.environ.get("ABL", "")


def build_nc(reps=1, loop_n=0):
    nc = bacc.Bacc("TRN2", target_bir_lowering=False, debug=False)
    x8d = nc.dram_tensor("x8", [C, NLOC], F8, kind="ExternalInput").ap()
    xTd = nc.dram_tensor("xT", [NLOC, C], BF, kind="ExternalInput").ap()
    cfd = nc.dram_tensor("cf8", [C, M], F8, kind="ExternalInput").ap()
    # packed weights (C, 416): [Wq4T | WkT | WvT] fp8, all pre-scaled x16
    wd = nc.dram_tensor("Wpack", [C, 416], F8, kind="ExternalInput").ap()
    # packed per-partition constants (128, 259): [bq4 | bk4 | gamma | bv_bc]
    bd = nc.dram_tensor("Bpack", [P, 259], F32, kind="ExternalInput").ap()
    o_dram = nc.dram_tensor("out", [NLOC, C], BF, kind="ExternalOutput").ap()

    with tile.TileContext(nc) as tc:
        if loop_n:
            # hardware loop: same body executed loop_n times (for timing)
            with tc.For_i(0, loop_n, 1):
                _emit(tc, x8d, xTd, cfd, wd, bd, o_dram)
        else:
            for _ in range(reps):
                _emit(tc, x8d, xTd, cfd, wd, bd, o_dram)
    nc.compile()
    return nc


def _emit(tc, x8d, xTd, cfd, wd, bd, o_dram):
    nc = tc.nc

    with ExitStack() as ctx:
        const = ctx.enter_context(tc.tile_pool(name="const", bufs=1))

        # ---- input DMAs: SP/DVE/Pool queues only (ACT stays exp-only) ----
        wall = const.tile([P, 2, 416], F8)       # [c_in_chunk, ci, col]
        nc.sync.dma_start(
            out=wall,
            in_=bass.AP(tensor=wd.tensor, offset=wd.offset,
                        ap=[[416, P], [416 * P, 2], [1, 416]]),
        )
        wq4 = wall[:, :, 0:P]
        wk = wall[:, :, P:P + RC]
        wv = wall[:, :, P + RC:416]
        cf8 = const.tile([P, 2, M], F8)

        def cf_dma(eng, c0, c1):
            eng.dma_start(
                out=cf8[:, :, c0:c1],
                in_=bass.AP(tensor=cfd.tensor, offset=cfd.offset + c0,
                            ap=[[M, P], [M * P, 2], [1, c1 - c0]]),
            )

        xf8 = const.tile([P, 2, NLOC], F8)       # [c_in_chunk, ci, n]

        def xf_dma(eng, a, b):
            eng.dma_start(
                out=xf8[:, :, a:b],
                in_=bass.AP(tensor=x8d.tensor, offset=x8d.offset + a,
                            ap=[[NLOC, P], [NLOC * P, 2], [1, b - a]]),
            )

        cf_dma(nc.sync, 0, 512)
        xf_dma(nc.sync, 0, 512)
        xf_dma(nc.sync, 512, NLOC)
        cf_dma(nc.sync, 1536, 2048)
        cf_dma(nc.sync, 3584, 4096)
        cf_dma(nc.scalar, 512, 1024)
        cf_dma(nc.scalar, 1024, 1536)
        bpack = const.tile([P, 259], F32)
        nc.gpsimd.dma_start(out=bpack, in_=bd)
        cf_dma(nc.gpsimd, 2048, 2560)
        cf_dma(nc.gpsimd, 2560, 3072)
        cf_dma(nc.gpsimd, 3072, 3584)
        bq4_sb = bpack[:, 0:1]
        bk4_sb = bpack[:, 1:2]
        gamma_bc = bpack[:, 2:3]
        bv_bc = bpack[:, 3:259]
        xbT = const.tile([P, NLOC // P, C], BF)  # [n_in_tile, nt, d]
        for tb in range(2):
            t0 = tb * 8
            nc.gpsimd.dma_start(
                out=xbT[:, t0:t0 + 8, :],
                in_=bass.AP(tensor=xTd.tensor, offset=xTd.offset + t0 * P * C,
                            ap=[[C, P], [P * C, 8], [1, C]]),
            )

        # gamma/16 (fp8 v-scale fold); xgv = xT + gamma*bv (epilogue adduct)
        gamma16 = const.tile([P, 1], F32)
        nc.vector.tensor_scalar_mul(gamma16, gamma_bc, 1.0 / WS)
        gvb = const.tile([P, C], F32)
        nc.gpsimd.tensor_scalar_mul(gvb, bv_bc, gamma_bc)
        xgv = const.tile([P, NLOC // P, C], F32)

        # ---- persistent SBUF operands -----------------------------------
        q4 = const.tile([P, NLOC], BF)           # q replicated on 4 part groups
        k4 = const.tile([P, NGRP, P], BF)        # m-tile 4g+i at partitions 32i
        vT = const.tile([P, MT, 272], F8)        # [m_in_tile, mt, d]; col 256=1
        nc.gpsimd.memset(vT[:, :, 257:258], 0.0)
        nc.gpsimd.memset(vT[:, :, 256:257], 1.0)

        ATTN_BUFS = int(os.environ.get("ATTN_BUFS", "14"))
        ATW = int(os.environ.get("ATW", "512"))
        attn = ctx.enter_context(tc.tile_pool(name="attn", bufs=ATTN_BUFS))
        eps = ctx.enter_context(tc.tile_pool(name="eps", bufs=4))
        outp = ctx.enter_context(tc.tile_pool(name="outp", bufs=2))

        at_tiles = {}
        res2_by_strip = {}
        at_static = []
        if "avstatic" in _ABL:
            ats0 = const.tile([P, 4, SW], F8, name="ats0")
            nc.gpsimd.memset(ats0, 0.002)
            at_static.append(ats0)

        with tc.tile_pool(name="psL", bufs=1, space="PSUM") as psL:

            pl_tiles = {}

            def logits_mms(T):
                # emitted one step AHEAD of its exp: the 3-tag rotation
                # makes slot (2T)%3,(2T+1)%3 disjoint from exp(T-1)'s
                # slots, so these never gate and sit before the att@v
                # chunks in the in-order PE queue
                s, g = divmod(T, NGRP)
                nsl = slice(s * SW, (s + 1) * SW)
                pls = []
                for h in range(2):
                    tg = (2 * T + h) % 3
                    pl = psL.tile([P, 2, SW], F32, tag=f"pl{tg}", name=f"pl{tg}")
                    for i in (2 * h, 2 * h + 1):
                        nc.tensor.matmul(
                            pl[:, i % 2, :],
                            k4[32 * i:32 * (i + 1), g, :],
                            q4[32 * i:32 * (i + 1), nsl],
                            start=True, stop=True,
                            tile_position=(32 * i, 0),
                        )
                    pls.append(pl)
                pl_tiles[T] = pls

            def exp_emit(T):
                pls = pl_tiles.pop(T)
                at = attn.tile([P, 4, ATW], F8, tag="at", name="at")
                for h in range(2):
                    nc.scalar.activation(
                        out=at[:, 2 * h:2 * h + 2, 0:SW], in_=pls[h][:, :, :],
                        func=mybir.ActivationFunctionType.Exp, scale=SCALE,
                    )
                at_tiles[T] = at

            def xgv_prep(T):
                t = T - 4
                if 0 <= t < NLOC // P:
                    nc.gpsimd.tensor_add(xgv[:, t, :], xbT[:, t, :], gvb)

            # ---- strip 0: projections interleaved with logits/exp -------
            with tc.tile_pool(name="psP", bufs=1, space="PSUM") as psP, \
                 tc.tile_pool(name="psV", bufs=1, space="PSUM") as psV:

                def qchunk(sq):
                    sl = slice(sq * 512, (sq + 1) * 512)
                    pq = psP.tile([P, 512], F32, tag="pp")
                    nc.tensor.matmul(
                        pq, wq4[:, :, :], xf8[:, :, sl],
                        start=True, stop=True,
                        perf_mode=mybir.MatmulPerfMode.DoubleRow,
                    )
                    nc.vector.tensor_scalar_add(q4[:, sl], pq, bq4_sb)

                def kproj(g):
                    # 4-way column-tiled: m-tile 4g+i -> partitions 32i
                    pk = psP.tile([P, 512], F32, tag="pp")
                    for i in range(4):
                        for ci in range(2):
                            msl = slice((4 * g + i) * P, (4 * g + i + 1) * P)
                            nc.tensor.matmul(
                                pk[32 * i:32 * (i + 1), 0:P],
                                wk[:, ci, :], cf8[:, ci, msl],
                                start=(ci == 0),
                                stop=(i == 3 and ci == 1),
                                skip_group_check=True,
                                tile_position=(0, 32 * i),
                            )
                    nc.vector.tensor_scalar_add(k4[:, g, :], pk[:, 0:P], bk4_sb)

                def vproj_half(g, half):
                    # two m-tiles per pv tile (2x256 cols in one bank);
                    # the two halves are emitted at opposite ends of a step
                    # so the PSUM-WAR on the single bank never stalls PE
                    pv = psV.tile([P, 512], F32, tag="pv")
                    mt0 = 4 * g + 2 * half
                    for q in range(2):
                        msl = slice((mt0 + q) * P, (mt0 + q + 1) * P)
                        nc.tensor.matmul(
                            pv[:, q * C:(q + 1) * C],
                            cf8[:, :, msl], wv[:, :, :],
                            start=True, stop=True,
                            perf_mode=mybir.MatmulPerfMode.DoubleRow,
                        )
                    # copies after both matmuls: DVE-read + PE-write of the
                    # same PSUM bank would otherwise be serialized (fatal on
                    # HW if concurrent)
                    for q in range(2):
                        nc.vector.tensor_copy(
                            vT[:, mt0 + q, 0:256], pv[:, q * C:(q + 1) * C]
                        )

                qchunk(0)
                kproj(0)
                vproj_half(0, 0)
                logits_mms(0)
                vproj_half(0, 1)
                for g in range(NGRP):
                    exp_emit(g)
                    if g < 3:
                        qchunk(g + 1)
                    if g < NGRP - 1:
                        kproj(g + 1)
                        vproj_half(g + 1, 0)
                    logits_mms(g + 1)
                    if g < NGRP - 1:
                        vproj_half(g + 1, 1)
                    xgv_prep(g)

            # ---- strips 1-3 + one-strip-behind att@v in a 2-bank pair ---
            with tc.tile_pool(name="psAV", bufs=1, space="PSUM") as psAV:
                av_cur = {}

                def av_chunk(s2, pair, g2):
                    # pair 0 -> n-tiles (0,1) of strip s2; pair 1 -> (2,3)
                    if g2 == 0:
                        av_cur["t"] = [
                            psAV.tile([P, 512], F32, tag=f"av{u}", name=f"av{u}")
                            for u in range(2)
                        ]
                    ats = at_tiles[s2 * NGRP + g2]
                    if pair == 1 and g2 == NGRP - 1:
                        del at_tiles[s2 * NGRP + g2]
                    if "avstatic" in _ABL:
                        ats = at_static[0]
                    for tl in range(2):
                        mtt = 4 * g2 + 2 * tl
                        for u in range(2):
                            j = 2 * pair + u
                            nc.tensor.matmul(
                                av_cur["t"][u][:, 0:258],
                                ats[:, 2 * tl:2 * tl + 2, j * P:(j + 1) * P],
                                vT[:, mtt:mtt + 2, 0:258],
                                start=(g2 == 0 and tl == 0),
                                stop=(g2 == NGRP - 1 and tl == 1),
                                perf_mode=mybir.MatmulPerfMode.DoubleRow,
                            )

                def epi_pair(s2, pair):
                    if "noepi" in _ABL:
                        sink = eps.tile([P, 2], F32, tag="sink", name="sink")
                        for u in range(2):
                            nc.vector.tensor_copy(
                                sink[:, u:u + 1], av_cur["t"][u][:, 256:257])
                        return
                    if pair == 0:
                        res2_by_strip[s2] = outp.tile([P, 4, C], BF, tag="res2", name="res2")
                    res2 = res2_by_strip[s2]
                    for u in range(2):
                        j = 2 * pair + u
                        t = 4 * s2 + j
                        rec = eps.tile([P, 1], F32, tag="rec", name="rec")
                        nc.vector.reciprocal(rec, av_cur["t"][u][:, 256:257])
                        recg = eps.tile([P, 1], F32, tag="recg", name="recg")
                        nc.vector.tensor_scalar_mul(recg, rec, gamma16)
                        nc.vector.scalar_tensor_tensor(
                            res2[:, j, :], av_cur["t"][u][:, 0:256], recg,
                            xgv[:, t, :], MULT, ADD,
                        )
                    if pair == 1:
                        nc.sync.dma_start(
                            out=bass.AP(
                                tensor=o_dram.tensor,
                                offset=o_dram.offset + 4 * s2 * P * C,
                                ap=[[C, P], [P * C, 4], [1, C]],
                            ),
                            in_=res2_by_strip.pop(s2),
                        )

                def av_slot(sA, w):
                    pair, ww = divmod(w, 4)
                    av_chunk(sA, pair, 2 * ww)
                    av_chunk(sA, pair, 2 * ww + 1)
                    if ww == 3:
                        epi_pair(sA, pair)

                for T in range(NGRP, NSTEP):
                    exp_emit(T)
                    if T < NSTEP - 1:
                        logits_mms(T + 1)
                    av_slot(T // NGRP - 1, T % NGRP)
                    xgv_prep(T)
                for w in range(NGRP):          # drain strip 3
                    av_slot(NSTRIPS - 1, w)


def _shard_inputs(x, context, Wq, bq, Wk, bk, Wv, bv, gamma):
    import ml_dtypes

    f8 = ml_dtypes.float8_e4m3
    bf16 = ml_dtypes.bfloat16
    xb = np.ascontiguousarray(np.asarray(x, dtype=np.float32)).reshape(B, C, NPIX)
    cb = np.ascontiguousarray(np.asarray(context, dtype=np.float32)).reshape(B, C, NPIX)
    WqT = np.asarray(Wq, dtype=np.float32).T * WS      # (C, RC)
    wpack = np.concatenate(
        [np.tile(WqT, (1, 4)),
         np.asarray(Wk, np.float32).T * WS,
         np.asarray(Wv, np.float32).T * WS], axis=1)
    bpack = np.concatenate(
        [np.tile(np.asarray(bq, np.float32) * WS, 4)[:, None],
         np.tile(np.asarray(bk, np.float32) * WS, 4)[:, None],
         np.full((P, 1), np.float32(np.asarray(gamma).reshape(-1)[0]), np.float32),
         np.tile(np.asarray(bv, np.float32)[None, :], (P, 1))], axis=1)
    shared = {
        "Wpack": np.ascontiguousarray(wpack.astype(f8)),
        "Bpack": np.ascontiguousarray(bpack.astype(np.float32)),
    }
    in_maps = []
    for core in range(N_CORES):
        b, half = core // 2, core % 2
        xs = xb[b][:, half * NLOC:(half + 1) * NLOC]
        m = dict(shared)
        m["x8"] = np.ascontiguousarray(xs.astype(f8))
        m["xT"] = np.ascontiguousarray(xs.T.astype(bf16))
        m["cf8"] = np.ascontiguousarray(cb[b].astype(f8))
        in_maps.append(m)
    return in_maps


def _gather(results):
    out = np.empty((B, C, NPIX), dtype=np.float32)
    for core in range(N_CORES):
        b, half = core // 2, core % 2
        out[b][:, half * NLOC:(half + 1) * NLOC] = (
            results[core]["out"].astype(np.float32).T
        )
    return out.reshape(B, C, H, W)


def run(inputs, trace=False, **kw):
    """Build (cached), run on the 8 NeuronCores, return (output, results)."""
    if "nc" not in _CACHE:
        _CACHE["nc"] = build_nc()
    nc = _CACHE["nc"]
    in_maps = _shard_inputs(**inputs)
    res = run_bass_kernel_spmd(
        nc, in_maps, core_ids=list(range(N_CORES)), trace=trace, **kw
    )
    return _gather(res.results), res


def kernel(**inputs) -> np.ndarray:
    out, _ = run(inputs, trace=False)
    return out
